# revision 3
# baseline (speedup 1.0000x reference)
"""Trainium2 Bass kernel for nn_DynamicAdapter (dense-MoE adapter block).

Math (per reference):
  pooled = mean_s(hidden)                               [B, H]
  gate = softmax(MLP_sel(MLP_ana(pooled)))              [B, E]
  h1_e = gelu(x @ W1_e + b1_e)                          [T, H/2]
  eo_e = (h1_e @ W2_e + b2_e) * gate[b, e]              [T, H]
  fused = sum_e eo_e @ Wf_e + f_b + x                   [T, H]
  out = layernorm(fused) * ln_g + ln_b

Key optimization vs the straightforward version: W2_e @ Wf_e is folded on
the host into a single Wc_e = W2_e @ Wf_e [F1, H] (plus cb_e = b2_e @ Wf_e),
so the expert path is two matmuls (x@W1_e, h1_e@Wc_e) instead of three —
half the tensor-engine MACs. Both matmuls run in fp8e4 with
perf_mode=DoubleRow (2 fp8 MACs per PE cell per cycle), for another ~2x.
Weights are pre-scaled by 64 on the host so fp8 values sit in the normal
range; the 1/64 is folded into the gelu input scale (mm1) and the gate
scale used during the fused accumulation (mm2).

Sharding: token-parallel. Core c handles tokens {(b, c*256+j)} — 1024 tokens.
Every core runs all 16 experts on its tokens (weights replicated), computes
the gate from per-core partial pools + a 16KB AllReduce, and writes its
token shard of the output.

Layout: x and h1 are feature-major ([feat_part, token_free]); mm2 uses h1
tiles as the stationary operand so fused comes out token-major, which makes
the residual+LayerNorm tail and output DMA natural. fp8 operands are stored
[P, k_tiles, free] so a [:, 2k:2k+2, :] slice is the 3D AP DoubleRow wants.
"""

import numpy as np
import ml_dtypes

import concourse.bacc as bacc
import concourse.mybir as mybir
import concourse.tile as tile
from concourse import bass_utils

BF16 = ml_dtypes.bfloat16
F8 = ml_dtypes.float8_e4m3

B, S, H, E = 4, 2048, 1024, 16
NCORES = 8
P = 128
TOK = B * S            # 8192 tokens total
TPC = TOK // NCORES    # 1024 tokens per core
SC = S // NCORES       # 256 tokens per (batch, core)
HT = H // P            # 8 h-tiles
F1 = H // 2            # 512 expert hidden
F1T = F1 // P          # 4 f1-tiles
TCH = 512              # moving-operand chunk (psum bank)
NCH = TPC // TCH       # 2 chunks
TT = TPC // P          # 8 token-tiles for mm2/tail

FP8 = True
W1SCALE = 64.0 if FP8 else 1.0
WCSCALE = 64.0 if FP8 else 1.0
PIPE = 3               # experts of mm1 in flight ahead of mm2

dt16 = mybir.dt.bfloat16
dt32 = mybir.dt.float32
dt8 = mybir.dt.float8e4
dte = dt8 if FP8 else dt16
AF = mybir.ActivationFunctionType
ALU = mybir.AluOpType
AX = mybir.AxisListType
PM = mybir.MatmulPerfMode.DoubleRow if FP8 else None
KG = 2 if FP8 else 1   # k-tiles consumed per matmul
NKG1 = HT // KG        # mm1 k-groups
NKG2 = F1T // KG       # mm2 k-groups

_BUILT = {}


def _build():
    if 0 in _BUILT:
        return _BUILT[0]

    nc = bacc.Bacc("TRN2", target_bir_lowering=False, debug=False)

    # ---- kernel I/O ----
    xq = nc.dram_tensor("xq", [HT, P, TPC], dte, kind="ExternalInput").ap()
    xres = nc.dram_tensor("xres", [TPC, H], dt32, kind="ExternalInput").ap()
    w1 = nc.dram_tensor("w1", [E, P, HT * F1], dte, kind="ExternalInput").ap()
    wc = nc.dram_tensor("wc", [E, P, F1T * H], dte, kind="ExternalInput").ap()
    b1d = nc.dram_tensor("b1d", [P, E * F1T], dt32, kind="ExternalInput").ap()
    cbd = nc.dram_tensor("cbd", [E, H], dt16, kind="ExternalInput").ap()
    a1 = nc.dram_tensor("a1", [HT, P, F1], dt16, kind="ExternalInput").ap()
    a2 = nc.dram_tensor("a2", [4, P, 256], dt16, kind="ExternalInput").ap()
    a3 = nc.dram_tensor("a3", [2, P, 128], dt16, kind="ExternalInput").ap()
    s1 = nc.dram_tensor("s1", [P, 64], dt16, kind="ExternalInput").ap()
    s2 = nc.dram_tensor("s2", [64, 32], dt16, kind="ExternalInput").ap()
    s3 = nc.dram_tensor("s3", [32, 16], dt16, kind="ExternalInput").ap()
    ab1 = nc.dram_tensor("ab1", [P, 4], dt32, kind="ExternalInput").ap()
    ab2 = nc.dram_tensor("ab2", [P, 2], dt32, kind="ExternalInput").ap()
    ab3 = nc.dram_tensor("ab3", [P, 1], dt32, kind="ExternalInput").ap()
    sb1 = nc.dram_tensor("sb1", [64, 1], dt32, kind="ExternalInput").ap()
    sb2 = nc.dram_tensor("sb2", [32, 1], dt32, kind="ExternalInput").ap()
    sb3 = nc.dram_tensor("sb3", [B, E], dt32, kind="ExternalInput").ap()
    fbbc_d = nc.dram_tensor("fbbc_d", [P, H], dt32, kind="ExternalInput").ap()
    gbc_d = nc.dram_tensor("gbc_d", [P, H], dt32, kind="ExternalInput").ap()
    bbc_d = nc.dram_tensor("bbc_d", [P, H], dt32, kind="ExternalInput").ap()
    out = nc.dram_tensor("out", [TPC, H], dt32, kind="ExternalOutput").ap()

    env = locals()
    with tile.TileContext(nc) as tc:
        _emit(tc, env)
    nc.compile()
    _BUILT[0] = nc
    return nc


def _emit(tc, t):
    nc = tc.nc
    with (
        tc.tile_pool(name="persist", bufs=1) as pp,
        tc.tile_pool(name="wpool", bufs=3) as wp,
        tc.tile_pool(name="hpool", bufs=PIPE + 1) as hp,
        tc.tile_pool(name="ps1p", bufs=4, space="PSUM") as ps1p,
    ):
        # ---------- critical-path DMAs first: x shard + early expert weights ----------
        x_all = pp.tile([P, HT, TPC], dte, name="x_all", tag="x_all")
        for i in range(HT):
            nc.sync.dma_start(out=x_all[:, i, :], in_=t["xq"][i])

        def fetch_weights(e):
            w1t = wp.tile([P, HT, F1], dte, name=f"w1t{e}", tag="w1t")
            src1 = t["w1"][e].rearrange("p (i f) -> p i f", i=HT)
            nc.sync.dma_start(out=w1t[:, 0:4, :], in_=src1[:, 0:4, :])
            nc.sync.dma_start(out=w1t[:, 4:8, :], in_=src1[:, 4:8, :])
            wct = wp.tile([P, F1T, H], dte, name=f"wct{e}", tag="wct")
            srcc = t["wc"][e].rearrange("p (m h) -> p m h", m=F1T)
            nc.sync.dma_start(out=wct[:, 0:2, :], in_=srcc[:, 0:2, :])
            nc.sync.dma_start(out=wct[:, 2:4, :], in_=srcc[:, 2:4, :])
            return w1t, wct

        w_cache = {e: fetch_weights(e) for e in range(PIPE)}
        b1_sb = pp.tile([P, E * F1T], dt32, name="b1_sb", tag="b1_sb")
        nc.sync.dma_start(out=b1_sb[:, :], in_=t["b1d"][:, :])
        fbbc = pp.tile([P, H], dt32, name="fbbc", tag="fbbc")
        nc.sync.dma_start(out=fbbc[:, :], in_=t["fbbc_d"][:, :])
        eps = pp.tile([P, 1], dt32, name="eps", tag="eps")
        nc.vector.memset(eps[:, :], 1e-5)

        fused = []
        for tau in range(TT):
            ft = pp.tile([P, H], dt32, name=f"fused{tau}", tag=f"fused{tau}")
            fused.append(ft)

        # per-(b,e) gate broadcast to all partitions (scaled by 1/WCSCALE)
        gate_sc = pp.tile([P, B * E], dt32, name="gate_sc", tag="gate_sc")
        # per-batch fused-bias broadcast: f_b + sum_e gate[b,e] * (b2_e @ Wf_e)
        cbg_bc = []
        for b in range(B):
            cb_t = pp.tile([P, H], dt32, name=f"cbg{b}", tag=f"cbg{b}")
            cbg_bc.append(cb_t)

        h1_cache = {}

        def emit_mm1(e):
            w1t, wct = w_cache.pop(e)
            h1t = hp.tile([P, F1T, TPC], dte, name=f"h1t{e}", tag="h1t")
            for m in range(F1T):
                pss = [
                    ps1p.tile([P, TCH], dt32, name="ps1", tag="ps1")
                    for _ in range(NCH)
                ]
                for g in range(NKG1):
                    for ch in range(NCH):
                        if FP8:
                            lhsT = w1t[:, KG * g : KG * (g + 1), m * P : (m + 1) * P]
                            rhs = x_all[:, KG * g : KG * (g + 1), ch * TCH : (ch + 1) * TCH]
                        else:
                            lhsT = w1t[:, g, m * P : (m + 1) * P]
                            rhs = x_all[:, g, ch * TCH : (ch + 1) * TCH]
                        nc.tensor.matmul(
                            pss[ch][:, :], lhsT, rhs,
                            start=(g == 0), stop=(g == NKG1 - 1), perf_mode=PM,
                        )
                for ch in range(NCH):
                    nc.scalar.activation(
                        h1t[:, m, ch * TCH : (ch + 1) * TCH], pss[ch][:, :], AF.Gelu,
                        bias=b1_sb[:, e * F1T + m : e * F1T + m + 1],
                        scale=1.0 / W1SCALE,
                    )
            h1_cache[e] = (h1t, wct)

        def emit_mm2(e, tail_cb=None):
            h1t, wct = h1_cache.pop(e)
            for tau in range(TT):
                b = tau // 2
                j = b * E + e
                pns = [
                    ps2p.tile([P, TCH], dt32, name="ps2", tag="ps2")
                    for _ in range(2)
                ]
                for g in range(NKG2):
                    for n in range(2):
                        if FP8:
                            lhsT = h1t[:, KG * g : KG * (g + 1), tau * P : (tau + 1) * P]
                            rhs = wct[:, KG * g : KG * (g + 1), n * TCH : (n + 1) * TCH]
                        else:
                            lhsT = h1t[:, g, tau * P : (tau + 1) * P]
                            rhs = wct[:, g, n * TCH : (n + 1) * TCH]
                        nc.tensor.matmul(
                            pns[n][:, :], lhsT, rhs,
                            start=(g == 0), stop=(g == NKG2 - 1), perf_mode=PM,
                        )
                for n in range(2):
                    dst = fused[tau][:, n * TCH : (n + 1) * TCH]
                    if e == 0:
                        nc.vector.tensor_scalar_mul(dst, pns[n][:, :], gate_sc[:, j : j + 1])
                    else:
                        nc.vector.scalar_tensor_tensor(
                            dst, pns[n][:, :], gate_sc[:, j : j + 1], dst,
                            op0=ALU.mult, op1=ALU.add,
                        )
                if tail_cb is not None:
                    tail_cb(tau)

        # ---------- phase 0a: pooling partials + AllReduce (no PE work) ----------
        with (
            tc.tile_pool(name="gw", bufs=1) as gw,
            tc.tile_pool(name="psgp", bufs=2, space="PSUM") as psgp,
            tc.tile_pool(name="cbps", bufs=2, space="PSUM") as cbp,
        ):
            # gate weights first: small, and needed as soon as pooling lands
            a1_sb = gw.tile([P, HT, F1], dt16, name="a1_sb", tag="a1_sb")
            nc.sync.dma_start(out=a1_sb[:, :, :], in_=t["a1"].rearrange("i p f -> p i f"))
            a2_sb = gw.tile([P, 4, 256], dt16, name="a2_sb", tag="a2_sb")
            nc.sync.dma_start(out=a2_sb[:, :, :], in_=t["a2"].rearrange("i p f -> p i f"))
            a3_sb = gw.tile([P, 2, 128], dt16, name="a3_sb", tag="a3_sb")
            nc.sync.dma_start(out=a3_sb[:, :, :], in_=t["a3"].rearrange("i p f -> p i f"))
            s1_sb = gw.tile([P, 64], dt16, name="s1_sb", tag="s1_sb")
            nc.sync.dma_start(out=s1_sb[:, :], in_=t["s1"][:, :])
            s2_sb = gw.tile([64, 32], dt16, name="s2_sb", tag="s2_sb")
            nc.sync.dma_start(out=s2_sb[:, :], in_=t["s2"][:, :])
            s3_sb = gw.tile([32, 16], dt16, name="s3_sb", tag="s3_sb")
            nc.sync.dma_start(out=s3_sb[:, :], in_=t["s3"][:, :])
            ab1_sb = gw.tile([P, 4], dt32, name="ab1_sb", tag="ab1_sb")
            nc.sync.dma_start(out=ab1_sb[:, :], in_=t["ab1"][:, :])
            ab2_sb = gw.tile([P, 2], dt32, name="ab2_sb", tag="ab2_sb")
            nc.sync.dma_start(out=ab2_sb[:, :], in_=t["ab2"][:, :])
            ab3_sb = gw.tile([P, 1], dt32, name="ab3_sb", tag="ab3_sb")
            nc.sync.dma_start(out=ab3_sb[:, :], in_=t["ab3"][:, :])
            sb1_sb = gw.tile([64, 1], dt32, name="sb1_sb", tag="sb1_sb")
            nc.sync.dma_start(out=sb1_sb[:, :], in_=t["sb1"][:, :])
            sb2_sb = gw.tile([32, 1], dt32, name="sb2_sb", tag="sb2_sb")
            nc.sync.dma_start(out=sb2_sb[:, :], in_=t["sb2"][:, :])
            sb3_sb = gw.tile([B, E], dt32, name="sb3_sb", tag="sb3_sb")
            nc.sync.dma_start(out=sb3_sb[:, :], in_=t["sb3"][:, :])
            cb_sb = gw.tile([E, H], dt16, name="cb_sb", tag="cb_sb")
            nc.sync.dma_start(out=cb_sb[:, :], in_=t["cbd"][:, :])

            # per-core partial pooling over own token shard, then a 16KB
            # AllReduce across the 8 cores.
            pooled_my = gw.tile([P, HT * B], dt32, name="pooled_my", tag="pooled_my")
            for i in range(HT):
                for b in range(B):
                    nc.vector.reduce_sum(
                        pooled_my[:, i * B + b : i * B + b + 1],
                        x_all[:, i, b * SC : (b + 1) * SC],
                        axis=AX.X,
                    )
            with tc.tile_pool(name="drac", bufs=1, space="DRAM") as dpc:
                arin = dpc.tile([P, HT * B], dt32, name="arin", tag="arin")
                arout = dpc.tile(
                    [P, HT * B], dt32, name="arout", tag="arout",
                    addr_space="Shared",
                )
                nc.sync.dma_start(out=arin[:, :], in_=pooled_my[:, :])
                nc.gpsimd.collective_compute(
                    "AllReduce",
                    ALU.add,
                    replica_groups=[list(range(NCORES))],
                    ins=[arin.opt()],
                    outs=[arout.opt()],
                )
                pooled_sum = gw.tile(
                    [P, HT * B], dt32, name="pooled_sum", tag="pooled_sum"
                )
                nc.sync.dma_start(out=pooled_sum[:, :], in_=arout[:, :])

            # PE work to hide the AllReduce + gate-MLP latency
            for e in range(min(PIPE, E)):
                emit_mm1(e)

            # ---------- phase 0b: gate MLP ----------
            ptb = []
            for i in range(HT):
                pb = gw.tile([P, B], dt16, name=f"ptb{i}", tag=f"ptb{i}")
                nc.scalar.mul(pb[:, :], pooled_sum[:, i * B : (i + 1) * B], 1.0 / S)
                ptb.append(pb)

            # gate MLP (feature-major)
            t1 = gw.tile([P, 16], dt16, name="t1", tag="t1")
            for m in range(4):
                psg = psgp.tile([P, B], dt32, name="psg1", tag="psg")
                for i in range(HT):
                    nc.tensor.matmul(
                        psg[:, :], a1_sb[:, i, m * P : (m + 1) * P], ptb[i][:, :],
                        start=(i == 0), stop=(i == HT - 1),
                    )
                nc.scalar.activation(
                    t1[:, m * B : (m + 1) * B], psg[:, :], AF.Gelu,
                    bias=ab1_sb[:, m : m + 1],
                )
            t2 = gw.tile([P, 8], dt16, name="t2", tag="t2")
            for m in range(2):
                psg = psgp.tile([P, B], dt32, name="psg2", tag="psg")
                for i in range(4):
                    nc.tensor.matmul(
                        psg[:, :], a2_sb[:, i, m * P : (m + 1) * P], t1[:, i * B : (i + 1) * B],
                        start=(i == 0), stop=(i == 3),
                    )
                nc.scalar.activation(
                    t2[:, m * B : (m + 1) * B], psg[:, :], AF.Gelu,
                    bias=ab2_sb[:, m : m + 1],
                )
            t3 = gw.tile([P, B], dt16, name="t3", tag="t3")
            psg = psgp.tile([P, B], dt32, name="psg3", tag="psg")
            for i in range(2):
                nc.tensor.matmul(
                    psg[:, :], a3_sb[:, i, :], t2[:, i * B : (i + 1) * B],
                    start=(i == 0), stop=(i == 1),
                )
            nc.scalar.activation(t3[:, :], psg[:, :], AF.Identity, bias=ab3_sb[:, 0:1])

            g1 = gw.tile([64, B], dt16, name="g1", tag="g1")
            psg = psgp.tile([64, B], dt32, name="psg4", tag="psg")
            nc.tensor.matmul(psg[:, :], s1_sb[:, :], t3[:, :], start=True, stop=True)
            nc.scalar.activation(g1[:, :], psg[:, :], AF.Gelu, bias=sb1_sb[:, 0:1])

            g2 = gw.tile([32, B], dt16, name="g2", tag="g2")
            psg = psgp.tile([32, B], dt32, name="psg5", tag="psg")
            nc.tensor.matmul(psg[:, :], s2_sb[:, :], g1[:, :], start=True, stop=True)
            nc.scalar.activation(g2[:, :], psg[:, :], AF.Gelu, bias=sb2_sb[:, 0:1])

            # flip to token-major: z[b, e]
            z = gw.tile([B, E], dt32, name="z", tag="z")
            psg = psgp.tile([B, E], dt32, name="psg6", tag="psg")
            nc.tensor.matmul(psg[:, :], g2[:, :], s3_sb[:, :], start=True, stop=True)
            nc.scalar.copy(z[:, :], psg[:, :])
            nc.vector.tensor_add(z[:, :], z[:, :], sb3_sb[:, :])

            # softmax over E (free dim)
            mx = gw.tile([B, 1], dt32, name="mx", tag="mx")
            nc.vector.reduce_max(mx[:, :], z[:, :], axis=AX.X)
            nc.vector.tensor_scalar_sub(z[:, :], z[:, :], mx[:, 0:1])
            sums = gw.tile([B, 1], dt32, name="sums", tag="sums")
            exps = gw.tile([B, E], dt32, name="exps", tag="exps")
            nc.scalar.activation(exps[:, :], z[:, :], AF.Exp, accum_out=sums[:, 0:1])
            rinv = gw.tile([B, 1], dt32, name="rinv", tag="rinv")
            nc.vector.reciprocal(rinv[:, :], sums[:, :])
            gate4 = gw.tile([B, E], dt32, name="gate4", tag="gate4")
            nc.vector.tensor_scalar_mul(gate4[:, :], exps[:, :], rinv[:, 0:1])

            # broadcast gate to all 128 partitions via DRAM bounce; also pull
            # back a transposed copy for the fused-bias matmul
            gate_bc = gw.tile([P, B * E], dt32, name="gate_bc", tag="gate_bc")
            gT_sb = gw.tile([E, B], dt32, name="gT_sb", tag="gT_sb")
            with tc.tile_pool(name="dramp", bufs=1, space="DRAM") as dp:
                gsc = dp.tile([1, B * E], dt32, name="gsc", tag="gsc")
                nc.sync.dma_start(
                    out=gsc.rearrange("o (b e) -> (o b) e", b=B), in_=gate4[:, :]
                )
                gflat = gw.tile([1, B * E], dt32, name="gflat", tag="gflat")
                nc.sync.dma_start(out=gflat[:, :], in_=gsc[:, :])
                nc.sync.dma_start(
                    out=gT_sb[:, :], in_=gsc.rearrange("o (b e) -> (o e) b", b=B)
                )
            nc.gpsimd.partition_broadcast(gate_bc[:, :], gflat[:, :])
            nc.vector.tensor_scalar_mul(gate_sc[:, :], gate_bc[:, :], 1.0 / WCSCALE)

            # cbg_bc[b] = f_b + sum_e gate[b,e] * cb_e, broadcast on all
            # partitions: lhsT[e, p] = gate[b, e] (replicated along free dim)
            ones16 = gw.tile([E, P], dt16, name="ones16", tag="ones16")
            nc.vector.memset(ones16[:, :], 1.0)
            for b in range(B):
                gud = gw.tile([E, P], dt16, name=f"gud{b}", tag="gud")
                nc.vector.tensor_scalar_mul(gud[:, :], ones16[:, :], gT_sb[:, b : b + 1])
                for n in range(2):
                    psc = cbp.tile([P, TCH], dt32, name="psc", tag="psc")
                    nc.tensor.matmul(
                        psc[:, :], gud[:, :], cb_sb[:, n * TCH : (n + 1) * TCH],
                        start=True, stop=True,
                    )
                    nc.vector.tensor_add(
                        cbg_bc[b][:, n * TCH : (n + 1) * TCH], psc[:, :],
                        fbbc[:, n * TCH : (n + 1) * TCH],
                    )

        # ---------- tail constants (needed only at the end) ----------
        gbc = pp.tile([P, H], dt32, name="gbc", tag="gbc")
        nc.sync.dma_start(out=gbc[:, :], in_=t["gbc_d"][:, :])
        bbc = pp.tile([P, H], dt32, name="bbc", tag="bbc")
        nc.sync.dma_start(out=bbc[:, :], in_=t["bbc_d"][:, :])

        # ---------- pools for the expert loop + interleaved tail ----------
        ps2p = tc.alloc_tile_pool(name="ps2p", bufs=4, space="PSUM")
        txf = tc.alloc_tile_pool(name="txf", bufs=5)
        tp = tc.alloc_tile_pool(name="tail", bufs=2)
        otp = tc.alloc_tile_pool(name="otp", bufs=2)
        sqp = tc.alloc_tile_pool(name="sqp", bufs=1)
        xrfs = {}

        def emit_xrf(tau):
            # residual + fused-bias prep; no expert-loop deps, runs on DVE slack
            xrf = txf.tile([P, H], dt32, name=f"xrf{tau}", tag="xrf")
            nc.sync.dma_start(out=xrf[:, :], in_=t["xres"][tau * P : (tau + 1) * P, :])
            nc.vector.tensor_add(xrf[:, :], xrf[:, :], cbg_bc[tau // 2][:, :])
            xrfs[tau] = xrf

        def emit_tail(tau):
            # layernorm tail for one token tile; GpSimd: +xrf, +ln_b;
            # DVE: reduce, scale*g; ACT: center, square-accum, sqrt.
            f2 = fused[tau]
            nc.gpsimd.tensor_add(f2[:, :], f2[:, :], xrfs[tau][:, :])
            ssum = tp.tile([P, 1], dt32, name="ssum", tag="ssum")
            nc.vector.reduce_sum(ssum[:, :], f2[:, :], axis=AX.X)
            negmu = tp.tile([P, 1], dt32, name="negmu", tag="negmu")
            nc.vector.tensor_scalar_mul(negmu[:, :], ssum[:, :], -1.0 / H)
            nc.scalar.activation(f2[:, :], f2[:, :], AF.Identity, bias=negmu[:, 0:1])
            sq = sqp.tile([P, H], dt16, name="sq", tag="sq")
            ssq = tp.tile([P, 1], dt32, name="ssq", tag="ssq")
            nc.scalar.activation(sq[:, :], f2[:, :], AF.Square, accum_out=ssq[:, 0:1])
            stdv = tp.tile([P, 1], dt32, name="stdv", tag="stdv")
            nc.scalar.activation(
                stdv[:, :], ssq[:, :], AF.Sqrt, scale=1.0 / H, bias=eps[:, 0:1]
            )
            rinv2 = tp.tile([P, 1], dt32, name="rinv2", tag="rinv2")
            nc.vector.reciprocal(rinv2[:, :], stdv[:, :])
            ot = otp.tile([P, H], dt32, name="ot", tag="ot")
            nc.vector.scalar_tensor_tensor(
                ot[:, :], f2[:, :], rinv2[:, 0:1], gbc[:, :],
                op0=ALU.mult, op1=ALU.mult,
            )
            nc.gpsimd.tensor_add(ot[:, :], ot[:, :], bbc[:, :])
            nc.sync.dma_start(out=t["out"][tau * P : (tau + 1) * P, :], in_=ot[:, :])

        # ---------- main expert loop: mm2(e) overlaps mm1(e+PIPE) ----------
        for e in range(E):
            if e == E - 3:
                for tau in range(5):
                    emit_xrf(tau)
            if e == E - 1:
                def tail_cb(tau):
                    if tau + 3 >= 5 and tau + 3 < TT:
                        emit_xrf(tau + 3)
                    emit_tail(tau)
            else:
                tail_cb = None
            emit_mm2(e, tail_cb)
            if e + PIPE < E:
                w_cache[e + PIPE] = fetch_weights(e + PIPE)
                emit_mm1(e + PIPE)
        sqp.release()
        otp.release()
        tp.release()
        txf.release()
        ps2p.release()


def _prep_inputs(inputs):
    """Host-side sharding/layout prep. Returns per-core input maps."""
    f32 = np.float32

    def bf(x):
        return np.ascontiguousarray(np.asarray(x, dtype=f32)).astype(BF16)

    def q8(x, scale):
        if not FP8:
            return bf(np.asarray(x, f32))
        v = np.asarray(x, f32) * scale
        return np.ascontiguousarray(np.clip(v, -240.0, 240.0)).astype(F8)

    hs = np.ascontiguousarray(np.asarray(inputs["hidden_states"], dtype=f32))  # [B,S,H]

    e1_w = np.asarray(inputs["e1_w"], f32)
    e1_b = np.asarray(inputs["e1_b"], f32)
    e2_w = np.asarray(inputs["e2_w"], f32)
    e2_b = np.asarray(inputs["e2_b"], f32)
    f_w = np.asarray(inputs["f_w"], f32)

    # fold W2 @ Wf into a single per-expert matrix (and its bias image)
    wc_f = np.matmul(e2_w, f_w)                        # [E, F1, H]
    cb = np.einsum("ef,efh->eh", e2_b, f_w)            # [E, H]

    w1q = q8(e1_w, W1SCALE)                            # [E, H, F1]
    wcq = q8(wc_f, WCSCALE)                            # [E, F1, H]

    common = {
        "w1": np.ascontiguousarray(
            w1q.reshape(E, HT, P, F1).transpose(0, 2, 1, 3)
        ).reshape(E, P, HT * F1),
        "wc": np.ascontiguousarray(
            wcq.reshape(E, F1T, P, H).transpose(0, 2, 1, 3)
        ).reshape(E, P, F1T * H),
        "b1d": np.ascontiguousarray(e1_b.reshape(E, F1T, P).transpose(2, 0, 1)).reshape(P, E * F1T),
        "cbd": bf(cb),
        "a1": bf(inputs["a1_w"]).reshape(HT, P, F1),
        "a2": bf(inputs["a2_w"]).reshape(4, P, 256),
        "a3": bf(inputs["a3_w"]).reshape(2, P, 128),
        "s1": bf(inputs["s1_w"]),
        "s2": bf(inputs["s2_w"]),
        "s3": bf(inputs["s3_w"]),
        "ab1": np.ascontiguousarray(np.asarray(inputs["a1_b"], f32).reshape(4, P).T),
        "ab2": np.ascontiguousarray(np.asarray(inputs["a2_b"], f32).reshape(2, P).T),
        "ab3": np.ascontiguousarray(np.asarray(inputs["a3_b"], f32).reshape(1, P).T),
        "sb1": np.ascontiguousarray(np.asarray(inputs["s1_b"], f32).reshape(64, 1)),
        "sb2": np.ascontiguousarray(np.asarray(inputs["s2_b"], f32).reshape(32, 1)),
        "sb3": np.ascontiguousarray(np.broadcast_to(np.asarray(inputs["s3_b"], f32), (B, E))),
        "fbbc_d": np.ascontiguousarray(np.broadcast_to(np.asarray(inputs["f_b"], f32), (P, H))),
        "gbc_d": np.ascontiguousarray(np.broadcast_to(np.asarray(inputs["ln_g"], f32), (P, H))),
        "bbc_d": np.ascontiguousarray(np.broadcast_to(np.asarray(inputs["ln_b"], f32), (P, H))),
    }

    in_maps = []
    for c in range(NCORES):
        shard = hs[:, c * SC : (c + 1) * SC, :]                     # [B, SC, H]
        xfm = np.ascontiguousarray(shard.transpose(2, 0, 1)).reshape(H, TPC)
        m = dict(common)
        m["xq"] = np.ascontiguousarray(q8(xfm, 1.0).reshape(HT, P, TPC))
        m["xres"] = np.ascontiguousarray(shard).reshape(TPC, H)
        in_maps.append(m)
    return in_maps


def kernel(**inputs) -> np.ndarray:
    nc = _build()
    in_maps = _prep_inputs(inputs)
    res = bass_utils.run_bass_kernel_spmd(nc, in_maps, core_ids=list(range(NCORES)))
    out_full = np.empty((B, S, H), dtype=np.float32)
    for c in range(NCORES):
        out_full[:, c * SC : (c + 1) * SC, :] = res.results[c]["out"].reshape(B, SC, H)
    return out_full


# revision 6
# speedup vs baseline: 1.3827x; 1.3827x over previous
"""Trainium2 Bass kernel for nn_DynamicAdapter (dense-MoE adapter block).

Math (per reference):
  pooled = mean_s(hidden)                               [B, H]
  gate = softmax(MLP_sel(MLP_ana(pooled)))              [B, E]
  h1_e = gelu(x @ W1_e + b1_e)                          [T, H/2]
  eo_e = (h1_e @ W2_e + b2_e) * gate[b, e]              [T, H]
  fused = sum_e eo_e @ Wf_e + f_b + x                   [T, H]
  out = layernorm(fused) * ln_g + ln_b

Optimizations vs the straightforward version:
  1. W2_e @ Wf_e folded on the host into Wc_e = W2_e @ Wf_e [F1, H] (plus
     cb_e = b2_e @ Wf_e), so the expert path is two matmuls instead of
     three — half the tensor-engine MACs.
  2. Both matmuls in fp8e4 with perf_mode=DoubleRow (2 fp8 MACs per PE
     cell per cycle). Weights pre-scaled by 64 on the host so fp8 values
     sit in the normal range.
  3. Two phases: phase 1 computes h1 for ALL 16 experts (hides the
     pooling AllReduce latency entirely); h1 is then scaled in place by
     8*gate[b,e] in fp8, so phase 2 can accumulate all 16 experts' fused
     contributions directly in PSUM (no per-expert DVE accumulate). The
     LayerNorm tail reads the final PSUM value directly and is spread
     evenly across phase 2's token tiles.

Sharding: token-parallel. Core c handles tokens {(b, c*256+j)} — 1024 tokens.
Every core runs all 16 experts on its tokens (weights replicated), computes
the gate from per-core partial pools + a 16KB AllReduce, and writes its
token shard of the output. fp8 operands are stored [P, k_tiles, free] so a
[:, 2k:2k+2, :] slice is the 3D AP DoubleRow wants.
"""

import numpy as np
import ml_dtypes

import concourse.bacc as bacc
import concourse.mybir as mybir
import concourse.tile as tile
from concourse import bass_utils

BF16 = ml_dtypes.bfloat16
F8 = ml_dtypes.float8_e4m3

B, S, H, E = 4, 2048, 1024, 16
NCORES = 8
P = 128
TOK = B * S            # 8192 tokens total
TPC = TOK // NCORES    # 1024 tokens per core
SC = S // NCORES       # 256 tokens per (batch, core)
HT = H // P            # 8 h-tiles
F1 = H // 2            # 512 expert hidden
F1T = F1 // P          # 4 f1-tiles
TCH = 512              # moving-operand chunk (psum bank)
NCH = TPC // TCH       # 2 chunks
TT = TPC // P          # 8 token-tiles for mm2/tail

FP8 = True
W1SCALE = 64.0 if FP8 else 1.0
WCSCALE = 64.0 if FP8 else 1.0
GSCALE = 8.0 if FP8 else 1.0            # gate pre-scale folded into h1
PSUM2SCALE = GSCALE * WCSCALE           # net scale on phase-2 psum

dt16 = mybir.dt.bfloat16
dt32 = mybir.dt.float32
dt8 = mybir.dt.float8e4
dte = dt8 if FP8 else dt16
AF = mybir.ActivationFunctionType
ALU = mybir.AluOpType
AX = mybir.AxisListType
PM = mybir.MatmulPerfMode.DoubleRow if FP8 else None
KG = 2 if FP8 else 1   # k-tiles consumed per matmul
NKG1 = HT // KG        # mm1 k-groups
NKG2 = F1T // KG       # mm2 k-groups

_BUILT = {}


def _build():
    if 0 in _BUILT:
        return _BUILT[0]

    nc = bacc.Bacc("TRN2", target_bir_lowering=False, debug=False)

    # ---- kernel I/O ----
    xq = nc.dram_tensor("xq", [HT, P, TPC], dte, kind="ExternalInput").ap()
    xres = nc.dram_tensor("xres", [TPC, H], dt32, kind="ExternalInput").ap()
    w1 = nc.dram_tensor("w1", [E, P, HT * F1], dte, kind="ExternalInput").ap()
    wc = nc.dram_tensor("wc", [E, P, F1T * H], dte, kind="ExternalInput").ap()
    b1d = nc.dram_tensor("b1d", [P, E * F1T], dt32, kind="ExternalInput").ap()
    cbd = nc.dram_tensor("cbd", [E, H], dt16, kind="ExternalInput").ap()
    a1 = nc.dram_tensor("a1", [HT, P, F1], dt16, kind="ExternalInput").ap()
    a2 = nc.dram_tensor("a2", [4, P, 256], dt16, kind="ExternalInput").ap()
    a3 = nc.dram_tensor("a3", [2, P, 128], dt16, kind="ExternalInput").ap()
    s1 = nc.dram_tensor("s1", [P, 64], dt16, kind="ExternalInput").ap()
    s2 = nc.dram_tensor("s2", [64, 32], dt16, kind="ExternalInput").ap()
    s3 = nc.dram_tensor("s3", [32, 16], dt16, kind="ExternalInput").ap()
    ab1 = nc.dram_tensor("ab1", [P, 4], dt32, kind="ExternalInput").ap()
    ab2 = nc.dram_tensor("ab2", [P, 2], dt32, kind="ExternalInput").ap()
    ab3 = nc.dram_tensor("ab3", [P, 1], dt32, kind="ExternalInput").ap()
    sb1 = nc.dram_tensor("sb1", [64, 1], dt32, kind="ExternalInput").ap()
    sb2 = nc.dram_tensor("sb2", [32, 1], dt32, kind="ExternalInput").ap()
    sb3 = nc.dram_tensor("sb3", [B, E], dt32, kind="ExternalInput").ap()
    fbbc_d = nc.dram_tensor("fbbc_d", [P, H], dt32, kind="ExternalInput").ap()
    gbc_d = nc.dram_tensor("gbc_d", [P, H], dt32, kind="ExternalInput").ap()
    bbc_d = nc.dram_tensor("bbc_d", [P, H], dt32, kind="ExternalInput").ap()
    out = nc.dram_tensor("out", [TPC, H], dt32, kind="ExternalOutput").ap()

    env = locals()
    with tile.TileContext(nc) as tc:
        _emit(tc, env)
    nc.compile()
    _BUILT[0] = nc
    return nc


def _emit(tc, t):
    nc = tc.nc
    with (
        tc.tile_pool(name="persist", bufs=1) as pp,
        tc.tile_pool(name="w1pool", bufs=3) as w1p,
    ):
        ps1p = tc.alloc_tile_pool(name="ps1p", bufs=4, space="PSUM")
        # ---------- critical-path DMAs first: x shard + early expert weights ----------
        x_all = pp.tile([P, HT, TPC], dte, name="x_all", tag="x_all")
        for i in range(HT):
            nc.sync.dma_start(out=x_all[:, i, :], in_=t["xq"][i])

        def fetch_w1(e):
            w1t = w1p.tile([P, HT, F1], dte, name=f"w1t{e}", tag="w1t")
            src1 = t["w1"][e].rearrange("p (i f) -> p i f", i=HT)
            nc.sync.dma_start(out=w1t[:, 0:4, :], in_=src1[:, 0:4, :])
            nc.sync.dma_start(out=w1t[:, 4:8, :], in_=src1[:, 4:8, :])
            return w1t

        def fetch_wc(e):
            wct = pp.tile([P, F1T, H], dte, name=f"wct{e}", tag=f"wct{e}")
            srcc = t["wc"][e].rearrange("p (m h) -> p m h", m=F1T)
            nc.sync.dma_start(out=wct[:, 0:2, :], in_=srcc[:, 0:2, :])
            nc.sync.dma_start(out=wct[:, 2:4, :], in_=srcc[:, 2:4, :])
            return wct

        w1_cache = {e: fetch_w1(e) for e in range(2)}
        b1_sb = pp.tile([P, E * F1T], dt32, name="b1_sb", tag="b1_sb")
        nc.sync.dma_start(out=b1_sb[:, :], in_=t["b1d"][:, :])
        fbbc = pp.tile([P, H], dt32, name="fbbc", tag="fbbc")
        nc.sync.dma_start(out=fbbc[:, :], in_=t["fbbc_d"][:, :])
        eps = pp.tile([P, 1], dt32, name="eps", tag="eps")
        nc.vector.memset(eps[:, :], 1e-5)

        # persistent per-expert tensors: h1 (scaled in place by 8*gate) + Wc
        h1s = [
            pp.tile([P, F1T, TPC], dte, name=f"h1s{e}", tag=f"h1s{e}")
            for e in range(E)
        ]
        wcts = {}

        # gate * 8 broadcast to all partitions (fp32)
        gate8 = pp.tile([P, B * E], dt32, name="gate8", tag="gate8")
        # per-batch fused-bias broadcast: f_b + sum_e gate[b,e] * (b2_e @ Wf_e)
        cbg_bc = []
        for b in range(B):
            cb_t = pp.tile([P, H], dt32, name=f"cbg{b}", tag=f"cbg{b}")
            cbg_bc.append(cb_t)

        def emit_mm1(e):
            w1t = w1_cache.pop(e)
            h1t = h1s[e]
            for m in range(F1T):
                pss = [
                    ps1p.tile([P, TCH], dt32, name="ps1", tag="ps1")
                    for _ in range(NCH)
                ]
                for g in range(NKG1):
                    for ch in range(NCH):
                        if FP8:
                            lhsT = w1t[:, KG * g : KG * (g + 1), m * P : (m + 1) * P]
                            rhs = x_all[:, KG * g : KG * (g + 1), ch * TCH : (ch + 1) * TCH]
                        else:
                            lhsT = w1t[:, g, m * P : (m + 1) * P]
                            rhs = x_all[:, g, ch * TCH : (ch + 1) * TCH]
                        nc.tensor.matmul(
                            pss[ch][:, :], lhsT, rhs,
                            start=(g == 0), stop=(g == NKG1 - 1), perf_mode=PM,
                        )
                for ch in range(NCH):
                    nc.scalar.activation(
                        h1t[:, m, ch * TCH : (ch + 1) * TCH], pss[ch][:, :], AF.Gelu,
                        bias=b1_sb[:, e * F1T + m : e * F1T + m + 1],
                        scale=1.0 / W1SCALE,
                    )

        def emit_h1scale(e):
            # in-place fp8 scale: h1s[e] *= 8 * gate[b, e] (per 256-token batch block)
            for b in range(B):
                j = b * E + e
                nc.vector.tensor_scalar_mul(
                    h1s[e][:, :, b * SC : (b + 1) * SC],
                    h1s[e][:, :, b * SC : (b + 1) * SC],
                    gate8[:, j : j + 1],
                )

        # ---------- phase 0a: pooling partials + AllReduce (no PE work) ----------
        with (
            tc.tile_pool(name="gw", bufs=1) as gw,
            tc.tile_pool(name="psgp", bufs=2, space="PSUM") as psgp,
            tc.tile_pool(name="cbps", bufs=2, space="PSUM") as cbp,
        ):
            # gate weights first: small, and needed as soon as pooling lands
            a1_sb = gw.tile([P, HT, F1], dt16, name="a1_sb", tag="a1_sb")
            nc.sync.dma_start(out=a1_sb[:, :, :], in_=t["a1"].rearrange("i p f -> p i f"))
            a2_sb = gw.tile([P, 4, 256], dt16, name="a2_sb", tag="a2_sb")
            nc.sync.dma_start(out=a2_sb[:, :, :], in_=t["a2"].rearrange("i p f -> p i f"))
            a3_sb = gw.tile([P, 2, 128], dt16, name="a3_sb", tag="a3_sb")
            nc.sync.dma_start(out=a3_sb[:, :, :], in_=t["a3"].rearrange("i p f -> p i f"))
            s1_sb = gw.tile([P, 64], dt16, name="s1_sb", tag="s1_sb")
            nc.sync.dma_start(out=s1_sb[:, :], in_=t["s1"][:, :])
            s2_sb = gw.tile([64, 32], dt16, name="s2_sb", tag="s2_sb")
            nc.sync.dma_start(out=s2_sb[:, :], in_=t["s2"][:, :])
            s3_sb = gw.tile([32, 16], dt16, name="s3_sb", tag="s3_sb")
            nc.sync.dma_start(out=s3_sb[:, :], in_=t["s3"][:, :])
            ab1_sb = gw.tile([P, 4], dt32, name="ab1_sb", tag="ab1_sb")
            nc.sync.dma_start(out=ab1_sb[:, :], in_=t["ab1"][:, :])
            ab2_sb = gw.tile([P, 2], dt32, name="ab2_sb", tag="ab2_sb")
            nc.sync.dma_start(out=ab2_sb[:, :], in_=t["ab2"][:, :])
            ab3_sb = gw.tile([P, 1], dt32, name="ab3_sb", tag="ab3_sb")
            nc.sync.dma_start(out=ab3_sb[:, :], in_=t["ab3"][:, :])
            sb1_sb = gw.tile([64, 1], dt32, name="sb1_sb", tag="sb1_sb")
            nc.sync.dma_start(out=sb1_sb[:, :], in_=t["sb1"][:, :])
            sb2_sb = gw.tile([32, 1], dt32, name="sb2_sb", tag="sb2_sb")
            nc.sync.dma_start(out=sb2_sb[:, :], in_=t["sb2"][:, :])
            sb3_sb = gw.tile([B, E], dt32, name="sb3_sb", tag="sb3_sb")
            nc.sync.dma_start(out=sb3_sb[:, :], in_=t["sb3"][:, :])
            cb_sb = gw.tile([E, H], dt16, name="cb_sb", tag="cb_sb")
            nc.sync.dma_start(out=cb_sb[:, :], in_=t["cbd"][:, :])

            # per-core partial pooling over own token shard, then a 16KB
            # AllReduce across the 8 cores.
            pooled_my = gw.tile([P, HT * B], dt32, name="pooled_my", tag="pooled_my")
            for i in range(HT):
                for b in range(B):
                    nc.vector.reduce_sum(
                        pooled_my[:, i * B + b : i * B + b + 1],
                        x_all[:, i, b * SC : (b + 1) * SC],
                        axis=AX.X,
                    )
            with tc.tile_pool(name="drac", bufs=1, space="DRAM") as dpc:
                arin = dpc.tile([P, HT * B], dt32, name="arin", tag="arin")
                arout = dpc.tile(
                    [P, HT * B], dt32, name="arout", tag="arout",
                    addr_space="Shared",
                )
                nc.sync.dma_start(out=arin[:, :], in_=pooled_my[:, :])
                nc.gpsimd.collective_compute(
                    "AllReduce",
                    ALU.add,
                    replica_groups=[list(range(NCORES))],
                    ins=[arin.opt()],
                    outs=[arout.opt()],
                )
                pooled_sum = gw.tile(
                    [P, HT * B], dt32, name="pooled_sum", tag="pooled_sum"
                )
                nc.sync.dma_start(out=pooled_sum[:, :], in_=arout[:, :])

            # PE work to hide the AllReduce latency: start on mm1
            w1_cache[2] = fetch_w1(2)
            wcts[0] = fetch_wc(0)
            emit_mm1(0)
            w1_cache[3] = fetch_w1(3)
            wcts[1] = fetch_wc(1)
            emit_mm1(1)

            # ---------- phase 0b: gate MLP ----------
            ptb = []
            for i in range(HT):
                pb = gw.tile([P, B], dt16, name=f"ptb{i}", tag=f"ptb{i}")
                nc.scalar.mul(pb[:, :], pooled_sum[:, i * B : (i + 1) * B], 1.0 / S)
                ptb.append(pb)

            # gate MLP (feature-major)
            t1 = gw.tile([P, 16], dt16, name="t1", tag="t1")
            for m in range(4):
                psg = psgp.tile([P, B], dt32, name="psg1", tag="psg")
                for i in range(HT):
                    nc.tensor.matmul(
                        psg[:, :], a1_sb[:, i, m * P : (m + 1) * P], ptb[i][:, :],
                        start=(i == 0), stop=(i == HT - 1),
                    )
                nc.scalar.activation(
                    t1[:, m * B : (m + 1) * B], psg[:, :], AF.Gelu,
                    bias=ab1_sb[:, m : m + 1],
                )
            t2 = gw.tile([P, 8], dt16, name="t2", tag="t2")
            for m in range(2):
                psg = psgp.tile([P, B], dt32, name="psg2", tag="psg")
                for i in range(4):
                    nc.tensor.matmul(
                        psg[:, :], a2_sb[:, i, m * P : (m + 1) * P], t1[:, i * B : (i + 1) * B],
                        start=(i == 0), stop=(i == 3),
                    )
                nc.scalar.activation(
                    t2[:, m * B : (m + 1) * B], psg[:, :], AF.Gelu,
                    bias=ab2_sb[:, m : m + 1],
                )
            t3 = gw.tile([P, B], dt16, name="t3", tag="t3")
            psg = psgp.tile([P, B], dt32, name="psg3", tag="psg")
            for i in range(2):
                nc.tensor.matmul(
                    psg[:, :], a3_sb[:, i, :], t2[:, i * B : (i + 1) * B],
                    start=(i == 0), stop=(i == 1),
                )
            nc.scalar.activation(t3[:, :], psg[:, :], AF.Identity, bias=ab3_sb[:, 0:1])

            g1 = gw.tile([64, B], dt16, name="g1", tag="g1")
            psg = psgp.tile([64, B], dt32, name="psg4", tag="psg")
            nc.tensor.matmul(psg[:, :], s1_sb[:, :], t3[:, :], start=True, stop=True)
            nc.scalar.activation(g1[:, :], psg[:, :], AF.Gelu, bias=sb1_sb[:, 0:1])

            g2 = gw.tile([32, B], dt16, name="g2", tag="g2")
            psg = psgp.tile([32, B], dt32, name="psg5", tag="psg")
            nc.tensor.matmul(psg[:, :], s2_sb[:, :], g1[:, :], start=True, stop=True)
            nc.scalar.activation(g2[:, :], psg[:, :], AF.Gelu, bias=sb2_sb[:, 0:1])

            # flip to token-major: z[b, e]
            z = gw.tile([B, E], dt32, name="z", tag="z")
            psg = psgp.tile([B, E], dt32, name="psg6", tag="psg")
            nc.tensor.matmul(psg[:, :], g2[:, :], s3_sb[:, :], start=True, stop=True)
            nc.scalar.copy(z[:, :], psg[:, :])
            nc.vector.tensor_add(z[:, :], z[:, :], sb3_sb[:, :])

            # softmax over E (free dim)
            mx = gw.tile([B, 1], dt32, name="mx", tag="mx")
            nc.vector.reduce_max(mx[:, :], z[:, :], axis=AX.X)
            nc.vector.tensor_scalar_sub(z[:, :], z[:, :], mx[:, 0:1])
            sums = gw.tile([B, 1], dt32, name="sums", tag="sums")
            exps = gw.tile([B, E], dt32, name="exps", tag="exps")
            nc.scalar.activation(exps[:, :], z[:, :], AF.Exp, accum_out=sums[:, 0:1])
            rinv = gw.tile([B, 1], dt32, name="rinv", tag="rinv")
            nc.vector.reciprocal(rinv[:, :], sums[:, :])
            gate4 = gw.tile([B, E], dt32, name="gate4", tag="gate4")
            nc.vector.tensor_scalar_mul(gate4[:, :], exps[:, :], rinv[:, 0:1])

            # broadcast gate to all 128 partitions via DRAM bounce; also pull
            # back a transposed copy for the fused-bias matmul
            gate_bc = gw.tile([P, B * E], dt32, name="gate_bc", tag="gate_bc")
            gT_sb = gw.tile([E, B], dt32, name="gT_sb", tag="gT_sb")
            with tc.tile_pool(name="dramp", bufs=1, space="DRAM") as dp:
                gsc = dp.tile([1, B * E], dt32, name="gsc", tag="gsc")
                nc.sync.dma_start(
                    out=gsc.rearrange("o (b e) -> (o b) e", b=B), in_=gate4[:, :]
                )
                gflat = gw.tile([1, B * E], dt32, name="gflat", tag="gflat")
                nc.sync.dma_start(out=gflat[:, :], in_=gsc[:, :])
                nc.sync.dma_start(
                    out=gT_sb[:, :], in_=gsc.rearrange("o (b e) -> (o e) b", b=B)
                )
            nc.gpsimd.partition_broadcast(gate_bc[:, :], gflat[:, :])
            nc.vector.tensor_scalar_mul(gate8[:, :], gate_bc[:, :], GSCALE)

            # cbg_bc[b] = f_b + sum_e gate[b,e] * cb_e, broadcast on all
            # partitions: lhsT[e, p] = gate[b, e] (replicated along free dim)
            ones16 = gw.tile([E, P], dt16, name="ones16", tag="ones16")
            nc.vector.memset(ones16[:, :], 1.0)
            for b in range(B):
                gud = gw.tile([E, P], dt16, name=f"gud{b}", tag="gud")
                nc.vector.tensor_scalar_mul(gud[:, :], ones16[:, :], gT_sb[:, b : b + 1])
                for n in range(2):
                    psc = cbp.tile([P, TCH], dt32, name="psc", tag="psc")
                    nc.tensor.matmul(
                        psc[:, :], gud[:, :], cb_sb[:, n * TCH : (n + 1) * TCH],
                        start=True, stop=True,
                    )
                    nc.vector.tensor_add(
                        cbg_bc[b][:, n * TCH : (n + 1) * TCH], psc[:, :],
                        fbbc[:, n * TCH : (n + 1) * TCH],
                    )

            # gate-scale the h1 tiles already in flight
            emit_h1scale(0)
            emit_h1scale(1)

            # ---------- rest of phase 1: mm1 for experts 2..15 ----------
            for e in range(2, E):
                if e + 2 < E:
                    w1_cache[e + 2] = fetch_w1(e + 2)
                wcts[e] = fetch_wc(e)
                emit_mm1(e)
                emit_h1scale(e)

        # ---------- tail constants (needed only at the end) ----------
        gbc = pp.tile([P, H], dt32, name="gbc", tag="gbc")
        nc.sync.dma_start(out=gbc[:, :], in_=t["gbc_d"][:, :])
        bbc = pp.tile([P, H], dt32, name="bbc", tag="bbc")
        nc.sync.dma_start(out=bbc[:, :], in_=t["bbc_d"][:, :])

        # ---------- phase 2 pools ----------
        ps1p.release()
        ps2p = tc.alloc_tile_pool(name="ps2p", bufs=6, space="PSUM")
        txf = tc.alloc_tile_pool(name="txf", bufs=3)
        tp = tc.alloc_tile_pool(name="tail", bufs=2)
        f2p = tc.alloc_tile_pool(name="f2p", bufs=3)
        sqp = tc.alloc_tile_pool(name="sqp", bufs=1)
        xrfs = {}

        def emit_xrf(tau):
            # residual + fused-bias prep; runs on DVE slack
            xrf = txf.tile([P, H], dt32, name=f"xrf{tau}", tag="xrf")
            nc.sync.dma_start(out=xrf[:, :], in_=t["xres"][tau * P : (tau + 1) * P, :])
            nc.vector.tensor_add(xrf[:, :], xrf[:, :], cbg_bc[tau // 2][:, :])
            xrfs[tau] = xrf

        def emit_tail(tau, pns):
            # f2 = psum/PSUM2SCALE + (xres + f_b + gate.cb); row-sums come for
            # free via accum_out. Then the layernorm chain.
            f2 = f2p.tile([P, H], dt32, name="f2", tag="f2")
            ss = tp.tile([P, 2], dt32, name="ss", tag="ss")
            for n in range(2):
                nc.vector.scalar_tensor_tensor(
                    f2[:, n * TCH : (n + 1) * TCH], pns[n][:, :], 1.0 / PSUM2SCALE,
                    xrfs[tau][:, n * TCH : (n + 1) * TCH],
                    op0=ALU.mult, op1=ALU.add,
                    accum_out=ss[:, n : n + 1],
                )
            ssum = tp.tile([P, 1], dt32, name="ssum", tag="ssum")
            nc.vector.tensor_add(ssum[:, :], ss[:, 0:1], ss[:, 1:2])
            negmu = tp.tile([P, 1], dt32, name="negmu", tag="negmu")
            nc.vector.tensor_scalar_mul(negmu[:, :], ssum[:, :], -1.0 / H)
            nc.scalar.activation(f2[:, :], f2[:, :], AF.Identity, bias=negmu[:, 0:1])
            sq = sqp.tile([P, H], dt16, name="sq", tag="sq")
            ssq = tp.tile([P, 1], dt32, name="ssq", tag="ssq")
            nc.scalar.activation(sq[:, :], f2[:, :], AF.Square, accum_out=ssq[:, 0:1])
            stdv = tp.tile([P, 1], dt32, name="stdv", tag="stdv")
            nc.scalar.activation(
                stdv[:, :], ssq[:, :], AF.Sqrt, scale=1.0 / H, bias=eps[:, 0:1]
            )
            rinv2 = tp.tile([P, 1], dt32, name="rinv2", tag="rinv2")
            nc.vector.reciprocal(rinv2[:, :], stdv[:, :])
            nc.vector.scalar_tensor_tensor(
                f2[:, :], f2[:, :], rinv2[:, 0:1], gbc[:, :],
                op0=ALU.mult, op1=ALU.mult,
            )
            nc.gpsimd.tensor_add(f2[:, :], f2[:, :], bbc[:, :])
            nc.sync.dma_start(out=t["out"][tau * P : (tau + 1) * P, :], in_=f2[:, :])

        # ---------- phase 2: per token tile, accumulate ALL experts in PSUM ----------
        emit_xrf(0)
        emit_xrf(1)
        for tau in range(TT):
            if tau + 2 < TT:
                emit_xrf(tau + 2)
            pns = [
                ps2p.tile([P, TCH], dt32, name="ps2", tag="ps2")
                for _ in range(2)
            ]
            for e in range(E):
                h1t = h1s[e]
                wct = wcts[e]
                for g in range(NKG2):
                    for n in range(2):
                        if FP8:
                            lhsT = h1t[:, KG * g : KG * (g + 1), tau * P : (tau + 1) * P]
                            rhs = wct[:, KG * g : KG * (g + 1), n * TCH : (n + 1) * TCH]
                        else:
                            lhsT = h1t[:, g, tau * P : (tau + 1) * P]
                            rhs = wct[:, g, n * TCH : (n + 1) * TCH]
                        nc.tensor.matmul(
                            pns[n][:, :], lhsT, rhs,
                            start=(e == 0 and g == 0),
                            stop=(e == E - 1 and g == NKG2 - 1),
                            perf_mode=PM,
                        )
            emit_tail(tau, pns)
        sqp.release()
        f2p.release()
        tp.release()
        txf.release()
        ps2p.release()


def _prep_inputs(inputs):
    """Host-side sharding/layout prep. Returns per-core input maps."""
    f32 = np.float32

    def bf(x):
        return np.ascontiguousarray(np.asarray(x, dtype=f32)).astype(BF16)

    def q8(x, scale):
        if not FP8:
            return bf(np.asarray(x, f32))
        v = np.asarray(x, f32) * scale
        return np.ascontiguousarray(np.clip(v, -240.0, 240.0)).astype(F8)

    hs = np.ascontiguousarray(np.asarray(inputs["hidden_states"], dtype=f32))  # [B,S,H]

    e1_w = np.asarray(inputs["e1_w"], f32)
    e1_b = np.asarray(inputs["e1_b"], f32)
    e2_w = np.asarray(inputs["e2_w"], f32)
    e2_b = np.asarray(inputs["e2_b"], f32)
    f_w = np.asarray(inputs["f_w"], f32)

    # fold W2 @ Wf into a single per-expert matrix (and its bias image)
    wc_f = np.matmul(e2_w, f_w)                        # [E, F1, H]
    cb = np.einsum("ef,efh->eh", e2_b, f_w)            # [E, H]

    w1q = q8(e1_w, W1SCALE)                            # [E, H, F1]
    wcq = q8(wc_f, WCSCALE)                            # [E, F1, H]

    common = {
        "w1": np.ascontiguousarray(
            w1q.reshape(E, HT, P, F1).transpose(0, 2, 1, 3)
        ).reshape(E, P, HT * F1),
        "wc": np.ascontiguousarray(
            wcq.reshape(E, F1T, P, H).transpose(0, 2, 1, 3)
        ).reshape(E, P, F1T * H),
        "b1d": np.ascontiguousarray(e1_b.reshape(E, F1T, P).transpose(2, 0, 1)).reshape(P, E * F1T),
        "cbd": bf(cb),
        "a1": bf(inputs["a1_w"]).reshape(HT, P, F1),
        "a2": bf(inputs["a2_w"]).reshape(4, P, 256),
        "a3": bf(inputs["a3_w"]).reshape(2, P, 128),
        "s1": bf(inputs["s1_w"]),
        "s2": bf(inputs["s2_w"]),
        "s3": bf(inputs["s3_w"]),
        "ab1": np.ascontiguousarray(np.asarray(inputs["a1_b"], f32).reshape(4, P).T),
        "ab2": np.ascontiguousarray(np.asarray(inputs["a2_b"], f32).reshape(2, P).T),
        "ab3": np.ascontiguousarray(np.asarray(inputs["a3_b"], f32).reshape(1, P).T),
        "sb1": np.ascontiguousarray(np.asarray(inputs["s1_b"], f32).reshape(64, 1)),
        "sb2": np.ascontiguousarray(np.asarray(inputs["s2_b"], f32).reshape(32, 1)),
        "sb3": np.ascontiguousarray(np.broadcast_to(np.asarray(inputs["s3_b"], f32), (B, E))),
        "fbbc_d": np.ascontiguousarray(np.broadcast_to(np.asarray(inputs["f_b"], f32), (P, H))),
        "gbc_d": np.ascontiguousarray(np.broadcast_to(np.asarray(inputs["ln_g"], f32), (P, H))),
        "bbc_d": np.ascontiguousarray(np.broadcast_to(np.asarray(inputs["ln_b"], f32), (P, H))),
    }

    in_maps = []
    for c in range(NCORES):
        shard = hs[:, c * SC : (c + 1) * SC, :]                     # [B, SC, H]
        xfm = np.ascontiguousarray(shard.transpose(2, 0, 1)).reshape(H, TPC)
        m = dict(common)
        m["xq"] = np.ascontiguousarray(q8(xfm, 1.0).reshape(HT, P, TPC))
        m["xres"] = np.ascontiguousarray(shard).reshape(TPC, H)
        in_maps.append(m)
    return in_maps


def kernel(**inputs) -> np.ndarray:
    nc = _build()
    in_maps = _prep_inputs(inputs)
    res = bass_utils.run_bass_kernel_spmd(nc, in_maps, core_ids=list(range(NCORES)))
    out_full = np.empty((B, S, H), dtype=np.float32)
    for c in range(NCORES):
        out_full[:, c * SC : (c + 1) * SC, :] = res.results[c]["out"].reshape(B, SC, H)
    return out_full
